# revision 49
# baseline (speedup 1.0000x reference)
"""Trainium2 Bass kernel for nn_BilinearHead (RMSNorm -> two 1x1 convs ->
bilinear scores at fixed index pairs + promo bias).

Math (per batch b):
    rms2[b]    = mean(x[b]**2) + eps
    f[b]       = from_w @ (x[b] * norm_weight) ;  t[b] = to_w @ (...)
    score[b,v] = <f[b,:,from_idx[v]], t[b,:,to_idx[v]]> / rms2[b]
                 + promo_bias[promo_idx[v]]
(valid because norm_weight == 1 and the conv biases are 0 for this problem's
input distribution; kernel() verifies and falls back to a host reference
otherwise).

Device algorithm (pure data parallel over batch: 8 cores x 128 batches),
all-fp16 on device:

  1. Host pre-packs x as fp16 [cp=128, b=128, par=2, hw=64] (4KB contiguous
     per partition per group DMA) and wpack as [cp, 4, 2, 128] (contiguous
     2KB/partition DMA -- a scattered layout here cost ~5us and gated the
     first GEMM in earlier versions).
  2. x group loads are spread across the sync/scalar HWDGE queues and the
     gpsimd SWDGE queue, all issued at the head so DMA streams continuously.
  3. Per batch-group of 16: one DVE square over [128, 2048], then the
     per-(cp,b) reduction split between GPSIMD (otherwise idle) and DVE.
  4. PE GEMM (fp16, parity-packed stacked weights): psum rows 0-63 =
     even-batch d, 64-127 = odd-batch d -> f, t in adjacent psum banks;
     single full-lane ACT evict.
  5. PE pair-packed Gt matmuls (quadrants (0,0)/(64,0), separate psum
     banks) -> ACT-evict to gt[64 j, 128 b, 64 i].
  6. After the loop: PE transpose z -> DVE reduce/scale/recip -> inv[b],
     overlapping the score matmuls.
  7. PE one-hot score matmuls per distinct from_idx value (columns sorted
     by from_idx); fused finalize per psum chunk on DVE
     (scalar_tensor_tensor: score * inv[b] + promo) -> fp16 -> DMA out.
  8. Host un-sorts columns and casts fp32.
"""

import sys

sys.path.insert(0, "/opt/trn_rl_repo")

import numpy as np

import concourse.bass as bass
import concourse.tile as tile
from concourse import mybir
from concourse.bacc import Bacc
from concourse.bass_utils import run_bass_kernel_spmd

# Problem shape (hardcoded per contest contract)
B_TOT, C, HW, D, V = 1024, 256, 64, 64, 1968
N_CORES = 8
B = B_TOT // N_CORES  # 128 batches per core
CP = C // 2  # 128 channel pairs (partition dim for GEMM)
NGROUPS = 8
GB = B // NGROUPS  # 16 batches per group
PAIRS_PER_GROUP = GB // 2
EPS = 1e-6
# score psum chunk boundaries (<=512 per bank); the last chunk is tiny so
# the final finalize+store tail after the last score matmul is short
CHUNK_BOUNDS = [0, 512, 1024, 1536, 1904, V]
F32 = mybir.dt.float32
F16 = mybir.dt.float16

# ---- engine-assignment knobs ----
# number of leading groups whose x^2 square runs on ACT (which is idle
# until the first GEMM's eviction); the rest go on DVE
ACT_SQ_G = 1
# split the last group's evictions across ACT and DVE (DVE is idle by the
# end of the loop) to cut the end-of-loop latency chain
SPLIT_LAST = False
# warmup matmuls (64-col) to lift the HAM clock gate while x0 loads; must
# bridge PE to the first GEMM with no idle gap or the clock ramp restarts
WARMUP_MM = 66
# keepalive matmuls between the last Gt and the score phase
KEEPALIVE_MM = 30


def build_kernel(seg_plan):
    """seg_plan: list of (i, col0, ncols) score-matmul segments, where i is
    the from_idx value, col0 the starting column in from_idx-sorted order,
    and the segment does not cross a 512 psum-bank boundary."""
    nc = Bacc()

    xs = nc.dram_tensor("xs", [CP, B, 2, HW], F16, kind="ExternalInput")
    # stacked conv weights, contiguous per partition:
    # [cp, 4 = (f_lo,f_hi,t_lo,t_hi), par, 128]
    wpack = nc.dram_tensor("wpack", [CP, 4, 2, 128], F16, kind="ExternalInput")
    ident = nc.dram_tensor("ident", [128, 128], F32, kind="ExternalInput")
    # one-hot(to) in from_idx-sorted column order (64 rows only; the old
    # 128-row zero-padded + promo-replicated "combo" wasted ~750KB of HBM
    # traffic on a DMA-bound head)
    onehot = nc.dram_tensor("onehot", [D, V], F16, kind="ExternalInput")
    # promo_bias[promo_idx] in sorted order, single row (broadcast on-chip)
    promo = nc.dram_tensor("promo", [1, V], F16, kind="ExternalInput")
    out = nc.dram_tensor("out", [B, V], F16, kind="ExternalOutput")

    with tile.TileContext(nc) as tc:
        with (
            tc.tile_pool(name="const", bufs=1) as const,
            tc.tile_pool(name="x2p", bufs=2) as x2p,
            tc.tile_pool(name="ft", bufs=2) as ftp,
            tc.tile_pool(name="psmm", bufs=2, space="PSUM") as psmm,
            tc.tile_pool(name="psgt", bufs=1, space="PSUM") as psgt,
            tc.tile_pool(name="pssc", bufs=2, space="PSUM") as pssc,
        ):
            # ---- persistent tiles ----
            xall = const.tile([CP, B, 2, HW], F16)  # all 8 groups
            wall = const.tile([CP, 4, 2, 128], F16)
            ident_sb = const.tile([128, 128], F32)
            oh_sb = const.tile([128, V], F16)
            promo_row = const.tile([1, V], F16)
            promo_bc = const.tile([128, V], F16)
            # [j, b, i]; rows 64:128 hold a DMA-duplicated copy so score
            # segments can run pairwise on both PE array halves
            gt_sb = const.tile([128, B, D], F16)
            z = const.tile([128, B], F32)  # [cp, b] partial x^2 sums
            final_sb = const.tile([128, V], F16)
            inv_sb = const.tile([128, 1], F32)

            # ---- head: issue every input DMA up front.
            # Only the two HWDGE queues (sync/scalar) move real bandwidth;
            # the gpsimd SWDGE queue is served last and must carry nothing
            # that paces the pipeline. x groups alternate sync/scalar so
            # arrivals come in natural order every ~1.7us.
            # each HWDGE queue sustains only ~100-170GB/s; evens ride the
            # sync queue, odds follow wall on the scalar queue, so group g
            # lands at roughly 11 + 2.3*g us -- just ahead of the loop.
            # sync: x0, x1, x2, x4, x6; scalar: wall then x3, x5, x7.
            # The odd groups front-load on the scalar queue so the x^2
            # chain (paced by pair-completing odd arrivals) never starves:
            # all of x lands by ~20us instead of ~24us.
            for g in (0, 1, 2, 4, 6):
                nc.sync.dma_start(
                    out=xall[:, g * GB : (g + 1) * GB, :, :],
                    in_=xs[:, g * GB : (g + 1) * GB, :, :],
                )
            nc.scalar.dma_start(out=wall, in_=wpack[:, :, :, :])
            for g in (3, 5, 7):
                nc.scalar.dma_start(
                    out=xall[:, g * GB : (g + 1) * GB, :, :],
                    in_=xs[:, g * GB : (g + 1) * GB, :, :],
                )
            nc.scalar.dma_start(out=oh_sb[0:D, :], in_=onehot[:, :])
            nc.scalar.dma_start(out=promo_row, in_=promo[:, :])
            nc.sync.dma_start(out=oh_sb[D:128, :], in_=oh_sb[0:D, :])
            nc.gpsimd.dma_start(out=ident_sb, in_=ident[:, :])
            # on-chip DMA broadcast of the promo row to all partitions
            # (finalize input, needed ~35us in; replaces 492KB of
            # replicated HBM traffic)
            nc.gpsimd.partition_broadcast(promo_bc[:, :], promo_row[:, :])

            # score psum chunks (column-partitioned; 2-buf rotation, so
            # chunk q+2 reuses chunk q's bank after its finalize)
            n_chunks = len(CHUNK_BOUNDS) - 1
            zt_ps = pssc.tile([128, 512], F32, tag="sc")  # z transpose target
            sc_ps = []
            for _q in range(n_chunks):
                sc_chunk = pssc.tile([128, 512], F32, tag="sc")
                sc_ps.append(sc_chunk)

            # PE warm-up burst while waiting for group 0's x: the HAM clock
            # gate needs ~3.4us of sustained matmul activity to lift the PE
            # from 1.2 to 2.4 GHz. A memset tile (no DMA dependency) lets
            # the burst start right after the preamble.
            wu_w = const.tile([128, 128], F16)
            nc.vector.memset(wu_w, 0.25)
            wu_ps = psgt.tile([D, 2, PAIRS_PER_GROUP, D], F32, tag="g2")
            for k in range(WARMUP_MM):
                nc.tensor.matmul(
                    out=wu_ps[:, 0, k % PAIRS_PER_GROUP, :],
                    lhsT=wu_w[:, 0:64],
                    rhs=wu_w[:, 64:128],
                    start=True,
                    stop=True,
                    tile_position=(0, 0),
                )

            # ---- x^2 pipeline, decoupled from the GEMM loop and paced by
            # x-group arrivals. DVE-only chain processing PAIRS of groups
            # per instruction (halves the per-op overhead; GPSIMD's folds
            # looked free but its SBUF traffic doubled DVE's square time).
            # ACT squares group 0 in its idle head window (its evictions
            # only start once the first GEMM finishes).
            PB = 2 * GB  # batches per pair
            def emit_sq_pair(k, defer_red=False):
                b0 = 2 * k * GB
                x2t = x2p.tile([128, PB, 2 * HW], F16, tag="x2", bufs=3)
                xflat = xall[:, b0 : b0 + PB, :, :].rearrange(
                    "p b par hw -> p b (par hw)"
                )
                if k == 0:
                    # group 0 on ACT (free until the first eviction),
                    # group 1 on DVE; both write halves of the pair tile
                    nc.scalar.activation(
                        out=x2t[:, 0:GB, :],
                        in_=xflat[:, 0:GB, :],
                        func=mybir.ActivationFunctionType.Square,
                    )
                    nc.vector.tensor_mul(
                        out=x2t[:, GB:PB, :],
                        in0=xflat[:, GB:PB, :],
                        in1=xflat[:, GB:PB, :],
                    )
                else:
                    nc.vector.tensor_mul(out=x2t[:, :, :], in0=xflat, in1=xflat)
                xh1 = x2p.tile([128, PB, HW], F16, tag="xh1", bufs=2)
                nc.vector.tensor_add(
                    out=xh1[:, :, :],
                    in0=x2t[:, :, 0:HW],
                    in1=x2t[:, :, HW : 2 * HW],
                )
                xh2 = x2p.tile([128, PB, HW // 2], F16, tag="xh2", bufs=2)
                nc.vector.tensor_add(
                    out=xh2[:, :, :],
                    in0=xh1[:, :, 0 : HW // 2],
                    in1=xh1[:, :, HW // 2 : HW],
                )
                if defer_red:
                    return xh2
                nc.vector.tensor_reduce(
                    out=z[:, b0 : b0 + PB],
                    in_=xh2[:, :, :],
                    axis=mybir.AxisListType.X,
                    op=mybir.AluOpType.add,
                )

            # the last pair's reduce is deferred until after the GEMM loop:
            # it then runs after DVE's share of the last evictions, so the
            # score phase is not gated on the x^2 chain's tail
            last_xh2 = None
            for k in range(NGROUPS // 2):
                last_xh2 = emit_sq_pair(k, defer_red=(k == NGROUPS // 2 - 1))

            # ---- main GEMM loop over batch groups (PE + ACT only).
            # GEMM is emitted one group ahead of Gt so the in-order PE queue
            # never stalls on ACT's ft eviction during pipeline fill.
            def emit_gemm(g, p0=0, npr=PAIRS_PER_GROUP):
                xv = xall[:, g * GB : (g + 1) * GB, :, :].rearrange(
                    "p (pr two) par hw -> p pr two par hw", two=2
                )[:, p0 : p0 + npr, :, :, :]
                ps2 = psmm.tile([128, 2, npr, HW], F32, tag="ps2")
                for fi, w0 in ((0, 0), (1, 2)):
                    for mi in range(4):
                        half, par0 = mi // 2, mi % 2
                        nc.tensor.matmul(
                            out=ps2[:, fi, :, :],
                            lhsT=wall[:, w0 + half, par0, :],
                            rhs=xv[:, :, half, par0, :],
                            start=(mi == 0),
                            stop=(mi == 3),
                        )
                ft_sb = ftp.tile([128, 2, npr, HW], F16, tag="ft")
                nc.scalar.copy(out=ft_sb[:, :, :, :], in_=ps2[:, :, :, :])
                return ft_sb

            def emit_gt(g, ft_sb, p0=0, npr=PAIRS_PER_GROUP, evict_dve=False):
                b0 = g * GB + 2 * p0
                # pair-packed Gt matmuls: Gt_b[j, i] = sum_d t[d,j] f[d,i]
                # The two row groups MUST write different psum banks:
                # concurrent row-tiled PE writes to one bank kill the HW run.
                # always allocate the full-size tile: a smaller one would
                # put the two quadrant row groups in the same psum bank,
                # and concurrent PE writes to one bank kill the HW run
                pgt2 = psgt.tile([D, 2, PAIRS_PER_GROUP, D], F32, tag="g2")
                for w in range(npr):
                    nc.tensor.matmul(
                        out=pgt2[:, 0, w, :],
                        lhsT=ft_sb[0:64, 1, w, :],
                        rhs=ft_sb[0:64, 0, w, :],
                        start=True,
                        stop=True,
                        tile_position=(0, 0),
                    )
                    nc.tensor.matmul(
                        out=pgt2[:, 1, w, :],
                        lhsT=ft_sb[64:128, 1, w, :],
                        rhs=ft_sb[64:128, 0, w, :],
                        start=True,
                        stop=True,
                        tile_position=(64, 0),
                    )
                # evict [j, (q, pair), i] -> gt[j, b, i], b = b0 + 2*pr + q
                # (contiguous 64-elem inner runs)
                ev = nc.vector.tensor_copy if evict_dve else nc.scalar.copy
                ev(
                    out=gt_sb[0:D, b0 : b0 + 2 * npr, :].rearrange(
                        "j (pr q) i -> j q pr i", q=2
                    ),
                    in_=pgt2[:, :, 0:npr, :],
                )
                # duplicate onto partitions 64:128 (idle sync queue) for
                # the double-pumped score phase
                nc.sync.dma_start(
                    out=gt_sb[D:128, b0 : b0 + 2 * npr, :],
                    in_=gt_sb[0:D, b0 : b0 + 2 * npr, :],
                )

            def keepalive(n, phase):
                # Dependency-free dummies that hold the HAM clock gate at
                # full speed while the PE waits on ACT's eviction chain.
                # They write unused columns of the zt tile (the only psum
                # slot with no pending rotation dependency).
                for k in range(n):
                    nc.tensor.matmul(
                        out=zt_ps[0:64, 128 + 64 * (k % 2) : 192 + 64 * (k % 2)],
                        lhsT=wu_w[:, 0:64],
                        rhs=wu_w[:, 64:128],
                        start=True,
                        stop=True,
                        tile_position=(0, 0),
                    )

            # group 7 is processed as two half-groups: the end-of-loop
            # serial chain (GEMM -> ft evict -> Gt -> gt evict -> score)
            # then only carries half a group's latency
            HP = PAIRS_PER_GROUP // 2
            fts = {}
            for g in range(NGROUPS - 1):
                fts[g] = emit_gemm(g)
                if g >= 1:
                    emit_gt(g - 1, fts[g - 1])
            ft7a = emit_gemm(NGROUPS - 1, 0, HP)
            emit_gt(NGROUPS - 2, fts[NGROUPS - 2])
            ft7b = emit_gemm(NGROUPS - 1, HP, HP)
            keepalive(KEEPALIVE_MM, 0)
            emit_gt(NGROUPS - 1, ft7a, 0, HP)
            emit_gt(NGROUPS - 1, ft7b, HP, HP)
            keepalive(44, 1)

            # deferred reduce of the last pair (after DVE's evict halves)
            nc.vector.tensor_reduce(
                out=z[:, B - PB : B],
                in_=last_xh2[:, :, :],
                axis=mybir.AxisListType.X,
                op=mybir.AluOpType.add,
            )

            # ---- one-hot score matmuls (columns in from_idx-sorted order).
            # Interleaved keepalive dummies hold the PE duty cycle high so
            # the HAM clock gate doesn't halve the clock mid-score (the
            # segments alone are LDWEIGHTS-paced at ~50% duty). They write
            # a fresh psgt-rotation tile, free once the last Gt eviction
            # has read its buffer.
            # segments grouped by chunk; hi=True runs a segment on PE
            # array rows 64:128 against the duplicated gt/onehot copies
            segs_by_chunk = [[] for _ in range(n_chunks)]
            for i, col0, ncols in seg_plan:
                q = next(
                    k for k in range(n_chunks) if CHUNK_BOUNDS[k + 1] > col0
                )
                segs_by_chunk[q].append((i, col0, ncols))

            def emit_seg(seg, hi):
                i, col0, ncols = seg
                q = next(
                    k for k in range(n_chunks) if CHUNK_BOUNDS[k + 1] > col0
                )
                c0 = col0 - CHUNK_BOUNDS[q]
                p0 = D if hi else 0
                nc.tensor.matmul(
                    out=sc_ps[q][:, c0 : c0 + ncols],
                    lhsT=gt_sb[p0 : p0 + D, :, i],
                    rhs=oh_sb[p0 : p0 + D, col0 : col0 + ncols],
                    start=True,
                    stop=True,
                    tile_position=(p0, 0),
                )

            def emit_fin(q):
                # fused finalize: out = score * inv[b] + promo_sorted
                q0 = CHUNK_BOUNDS[q]
                n = CHUNK_BOUNDS[q + 1] - q0
                nc.vector.scalar_tensor_tensor(
                    out=final_sb[:, q0 : q0 + n],
                    in0=sc_ps[q][:, 0:n],
                    scalar=inv_sb[:, 0:1],
                    in1=promo_bc[:, q0 : q0 + n],
                    op0=mybir.AluOpType.mult,
                    op1=mybir.AluOpType.add,
                )
                # per-chunk store so the DMA overlaps later chunks' finalize
                dma_eng = nc.sync if q % 2 == 0 else nc.scalar
                dma_eng.dma_start(
                    out=out[:, q0 : q0 + n],
                    in_=final_sb[:, q0 : q0 + n],
                )

            # chunk-0 segments first (single-pumped): they need only
            # gt + onehot, so they start right after the last Gt eviction
            # while the x^2 chain's tail (red-3) is still draining on DVE
            for seg in segs_by_chunk[0]:
                emit_seg(seg, False)

            # ---- 1/rms2 per batch (overlaps chunk 1+ score matmuls) ----
            # (transpose outputs must land at psum partition 0, so this
            # cannot be done incrementally per batch-slice)
            nc.tensor.transpose(out=zt_ps[:, 0:128], in_=z[:, :], identity=ident_sb[:, :])
            nc.vector.tensor_reduce(
                out=inv_sb[:, :],
                in_=zt_ps[:, 0:128],
                axis=mybir.AxisListType.X,
                op=mybir.AluOpType.add,
            )
            nc.vector.tensor_scalar(
                out=inv_sb[:, :],
                in0=inv_sb[:, :],
                scalar1=1.0 / (C * HW),
                scalar2=EPS,
                op0=mybir.AluOpType.mult,
                op1=mybir.AluOpType.add,
            )
            nc.vector.reciprocal(out=inv_sb[:, :], in_=inv_sb[:, :])
            emit_fin(0)

            # remaining chunks double-pumped in bank pairs: (1,2), (3,4).
            # The lo/hi streams write different psum banks, so the two PE
            # array halves run concurrently (same trick as the Gt matmuls).
            for qa, qb in ((1, 2), (3, 4)):
                la, lb = segs_by_chunk[qa], segs_by_chunk[qb]
                for k in range(max(len(la), len(lb))):
                    if k < len(la):
                        emit_seg(la[k], False)
                    if k < len(lb):
                        emit_seg(lb[k], True)
                emit_fin(qa)
                emit_fin(qb)

    nc.compile()
    return nc


_NC_CACHE = {}


def _plan_from_indices(from_idx, to_idx):
    from_idx = np.asarray(from_idx, np.int64)
    to_idx = np.asarray(to_idx, np.int64)
    order = np.argsort(from_idx, kind="stable")
    fi_sorted = from_idx[order]
    seg_plan = []
    col = 0
    for i in range(HW):
        n = int(np.count_nonzero(fi_sorted == i))
        while n > 0:
            # segments must not cross a psum chunk boundary
            bound = next(b for b in CHUNK_BOUNDS[1:] if b > col)
            m = min(n, bound - col)
            seg_plan.append((i, col, m))
            col += m
            n -= m
    assert col == V
    onehot = np.zeros((D, V), np.float16)
    onehot[to_idx[order], np.arange(V)] = 1.0
    return tuple(seg_plan), onehot, order


def _host_inputs(from_w, to_w):
    def stack_w(wmat):
        wt = np.ascontiguousarray(wmat.T).reshape(CP, 2, D)  # [cp, par, d]
        lo = np.zeros((2, CP, 128), np.float16)
        hi = np.zeros((2, CP, 128), np.float16)
        lo[:, :, 0:D] = wt.transpose(1, 0, 2)
        hi[:, :, D:128] = wt.transpose(1, 0, 2)
        return lo, hi

    wf_lo, wf_hi = stack_w(np.asarray(from_w, np.float32))
    wt_lo, wt_hi = stack_w(np.asarray(to_w, np.float32))
    return wf_lo, wf_hi, wt_lo, wt_hi


def _device_inputs(x, from_w, to_w, promo_bias, from_idx, to_idx, promo_idx):
    """Build (seg_plan, shared input map, per-core xs list, unsort order)."""
    seg_plan, onehot, order = _plan_from_indices(from_idx, to_idx)
    wf_lo, wf_hi, wt_lo, wt_hi = _host_inputs(from_w, to_w)
    # [4, 2, CP, 128] -> [CP, 4, 2, 128] contiguous so the upload DMA is
    # one 2KB descriptor per partition
    wpack = np.ascontiguousarray(
        np.stack([wf_lo, wf_hi, wt_lo, wt_hi], axis=0).transpose(2, 0, 1, 3)
    )
    promo = np.asarray(promo_bias, np.float32)[np.asarray(promo_idx, np.int64)]
    shared = {
        "wpack": wpack,
        "ident": np.eye(128, dtype=np.float32),
        "onehot": np.ascontiguousarray(onehot),
        "promo": promo[order].astype(np.float16)[None, :],
    }
    # x [B_TOT, C, HW] -> per-core [cp, b, par, hw] fp16 (4KB contiguous
    # per partition per group DMA)
    xr = np.asarray(x, np.float32).reshape(B_TOT, C, HW)
    xs_list = []
    for c in range(N_CORES):
        xc = xr[c * B : (c + 1) * B].reshape(B, CP, 2, HW)
        xs_list.append(np.ascontiguousarray(xc.transpose(1, 0, 2, 3)).astype(np.float16))
    return seg_plan, shared, xs_list, order


def kernel(
    x,
    norm_weight,
    from_w,
    from_b,
    to_w,
    to_b,
    promo_bias,
    from_idx,
    to_idx,
    promo_idx,
):
    x = np.asarray(x, np.float32)
    norm_weight = np.asarray(norm_weight, np.float32)
    from_b = np.asarray(from_b, np.float32)
    to_b = np.asarray(to_b, np.float32)

    if (
        np.any(from_b != 0.0)
        or np.any(to_b != 0.0)
        or not np.allclose(norm_weight, 1.0)
    ):
        # General-correctness fallback; never hit for this problem's input
        # distribution (norm_weight is ones, conv biases are zeros).
        return _host_reference(
            x, norm_weight, from_w, from_b, to_w, to_b, promo_bias,
            from_idx, to_idx, promo_idx,
        )

    seg_plan, shared, xs_list, order = _device_inputs(
        x, from_w, to_w, promo_bias, from_idx, to_idx, promo_idx
    )
    if seg_plan not in _NC_CACHE:
        _NC_CACHE[seg_plan] = build_kernel(seg_plan)
    nc = _NC_CACHE[seg_plan]

    in_maps = [dict(shared, xs=xs_list[c]) for c in range(N_CORES)]
    res = run_bass_kernel_spmd(nc, in_maps, core_ids=list(range(N_CORES)))
    full = np.empty((B_TOT, V), np.float32)
    for c in range(N_CORES):
        dev = np.asarray(res.results[c]["out"], np.float32)  # sorted columns
        full[c * B : (c + 1) * B, order] = dev
    return full


def _host_reference(
    x, norm_weight, from_w, from_b, to_w, to_b, promo_bias, from_idx, to_idx, promo_idx
):
    b, c, w, h = x.shape
    rms = np.sqrt(np.mean(x * x, axis=(1, 2, 3), keepdims=True) + EPS)
    xn = (x / rms) * norm_weight[None]
    f = (
        np.einsum("bchw,dc->bdhw", xn, from_w) + from_b[None, :, None, None]
    ).reshape(b, -1, w * h)
    t = (
        np.einsum("bchw,dc->bdhw", xn, to_w) + to_b[None, :, None, None]
    ).reshape(b, -1, w * h)
    score = np.einsum("bdv,bdv->bv", f[:, :, from_idx], t[:, :, to_idx])
    return (score + promo_bias[promo_idx][None, :]).astype(np.float32)


# revision 50
# speedup vs baseline: 1.0554x; 1.0554x over previous
"""Trainium2 Bass kernel for nn_BilinearHead (RMSNorm -> two 1x1 convs ->
bilinear scores at fixed index pairs + promo bias).

Math (per batch b):
    rms2[b]    = mean(x[b]**2) + eps
    f[b]       = from_w @ (x[b] * norm_weight) ;  t[b] = to_w @ (...)
    score[b,v] = <f[b,:,from_idx[v]], t[b,:,to_idx[v]]> / rms2[b]
                 + promo_bias[promo_idx[v]]
(valid because norm_weight == 1 and the conv biases are 0 for this problem's
input distribution; kernel() verifies and falls back to a host reference
otherwise).

Device algorithm (pure data parallel over batch: 8 cores x 128 batches),
all-fp16 on device:

  1. Host pre-packs x as fp16 [cp=128, b=128, par=2, hw=64] (4KB contiguous
     per partition per group DMA) and wpack as [cp, 4, 2, 128] (contiguous
     2KB/partition DMA -- a scattered layout here cost ~5us and gated the
     first GEMM in earlier versions).
  2. x group loads are spread across the sync/scalar HWDGE queues and the
     gpsimd SWDGE queue, all issued at the head so DMA streams continuously.
  3. Per batch-group of 16: one DVE square over [128, 2048], then the
     per-(cp,b) reduction split between GPSIMD (otherwise idle) and DVE.
  4. PE GEMM (fp16, parity-packed stacked weights): psum rows 0-63 =
     even-batch d, 64-127 = odd-batch d -> f, t in adjacent psum banks;
     single full-lane ACT evict.
  5. PE pair-packed Gt matmuls (quadrants (0,0)/(64,0), separate psum
     banks) -> ACT-evict to gt[64 j, 128 b, 64 i].
  6. After the loop: PE transpose z -> DVE reduce/scale/recip -> inv[b],
     overlapping the score matmuls.
  7. PE one-hot score matmuls per distinct from_idx value (columns sorted
     by from_idx); fused finalize per psum chunk on DVE
     (scalar_tensor_tensor: score * inv[b] + promo) -> fp16 -> DMA out.
  8. Host un-sorts columns and casts fp32.
"""

import sys

sys.path.insert(0, "/opt/trn_rl_repo")

import numpy as np

import concourse.bass as bass
import concourse.tile as tile
from concourse import mybir
from concourse.bacc import Bacc
from concourse.bass_utils import run_bass_kernel_spmd

# Problem shape (hardcoded per contest contract)
B_TOT, C, HW, D, V = 1024, 256, 64, 64, 1968
N_CORES = 8
B = B_TOT // N_CORES  # 128 batches per core
CP = C // 2  # 128 channel pairs (partition dim for GEMM)
NGROUPS = 8
GB = B // NGROUPS  # 16 batches per group
PAIRS_PER_GROUP = GB // 2
EPS = 1e-6
# score psum chunk boundaries (<=512 per bank); the last chunk is tiny so
# the final finalize+store tail after the last score matmul is short
CHUNK_BOUNDS = [0, 512, 1024, 1536, 1904, V]
F32 = mybir.dt.float32
F16 = mybir.dt.float16

# ---- engine-assignment knobs ----
# number of leading groups whose x^2 square runs on ACT (which is idle
# until the first GEMM's eviction); the rest go on DVE
ACT_SQ_G = 1
# split the last group's evictions across ACT and DVE (DVE is idle by the
# end of the loop) to cut the end-of-loop latency chain
SPLIT_LAST = False
# warmup matmuls (64-col) to lift the HAM clock gate while x0 loads; must
# bridge PE to the first GEMM with no idle gap or the clock ramp restarts
WARMUP_MM = 66
# keepalive matmuls between the last Gt and the score phase
KEEPALIVE_MM = 30


def build_kernel(seg_plan):
    """seg_plan: list of (i, col0, ncols) score-matmul segments, where i is
    the from_idx value, col0 the starting column in from_idx-sorted order,
    and the segment does not cross a 512 psum-bank boundary."""
    nc = Bacc()

    xs = nc.dram_tensor("xs", [CP, B, 2, HW], F16, kind="ExternalInput")
    # stacked conv weights, contiguous per partition:
    # [cp, 4 = (f_lo,f_hi,t_lo,t_hi), par, 128]
    wpack = nc.dram_tensor("wpack", [CP, 4, 2, 128], F16, kind="ExternalInput")
    ident = nc.dram_tensor("ident", [128, 128], F32, kind="ExternalInput")
    # one-hot(to) in from_idx-sorted column order (64 rows only; the old
    # 128-row zero-padded + promo-replicated "combo" wasted ~750KB of HBM
    # traffic on a DMA-bound head)
    onehot = nc.dram_tensor("onehot", [D, V], F16, kind="ExternalInput")
    # promo_bias[promo_idx] in sorted order, single row (broadcast on-chip)
    promo = nc.dram_tensor("promo", [1, V], F16, kind="ExternalInput")
    out = nc.dram_tensor("out", [B, V], F16, kind="ExternalOutput")

    with tile.TileContext(nc) as tc:
        with (
            tc.tile_pool(name="const", bufs=1) as const,
            tc.tile_pool(name="x2p", bufs=2) as x2p,
            tc.tile_pool(name="ft", bufs=2) as ftp,
            tc.tile_pool(name="psmm", bufs=2, space="PSUM") as psmm,
            tc.tile_pool(name="psgt", bufs=1, space="PSUM") as psgt,
            tc.tile_pool(name="pssc", bufs=2, space="PSUM") as pssc,
        ):
            # ---- persistent tiles ----
            xall = const.tile([CP, B, 2, HW], F16)  # all 8 groups
            wall = const.tile([CP, 4, 2, 128], F16)
            ident_sb = const.tile([128, 128], F32)
            oh_sb = const.tile([128, V], F16)
            promo_row = const.tile([1, V], F16)
            promo_bc = const.tile([128, V], F16)
            # [j, b, i]; rows 64:128 hold a DMA-duplicated copy so score
            # segments can run pairwise on both PE array halves
            gt_sb = const.tile([128, B, D], F16)
            z = const.tile([128, B], F32)  # [cp, b] partial x^2 sums
            final_sb = const.tile([128, V], F16)
            inv_sb = const.tile([128, 1], F32)

            # ---- head: issue every input DMA up front.
            # Only the two HWDGE queues (sync/scalar) move real bandwidth;
            # the gpsimd SWDGE queue is served last and must carry nothing
            # that paces the pipeline. x groups alternate sync/scalar so
            # arrivals come in natural order every ~1.7us.
            # each HWDGE queue sustains only ~100-170GB/s; evens ride the
            # sync queue, odds follow wall on the scalar queue, so group g
            # lands at roughly 11 + 2.3*g us -- just ahead of the loop.
            # sync: x0, x1, x2, x4, x6; scalar: wall then x3, x5, x7.
            # The odd groups front-load on the scalar queue so the x^2
            # chain (paced by pair-completing odd arrivals) never starves:
            # all of x lands by ~20us instead of ~24us.
            for g in (0, 1, 2, 4, 6):
                nc.sync.dma_start(
                    out=xall[:, g * GB : (g + 1) * GB, :, :],
                    in_=xs[:, g * GB : (g + 1) * GB, :, :],
                )
            nc.scalar.dma_start(out=wall, in_=wpack[:, :, :, :])
            for g in (3, 5, 7):
                nc.scalar.dma_start(
                    out=xall[:, g * GB : (g + 1) * GB, :, :],
                    in_=xs[:, g * GB : (g + 1) * GB, :, :],
                )
            nc.scalar.dma_start(out=oh_sb[0:D, :], in_=onehot[:, :])
            nc.scalar.dma_start(out=promo_row, in_=promo[:, :])
            nc.sync.dma_start(out=oh_sb[D:128, :], in_=oh_sb[0:D, :])
            nc.gpsimd.dma_start(out=ident_sb, in_=ident[:, :])
            # on-chip DMA broadcast of the promo row to all partitions
            # (finalize input, needed ~35us in; replaces 492KB of
            # replicated HBM traffic)
            nc.gpsimd.partition_broadcast(promo_bc[:, :], promo_row[:, :])

            # score psum chunks (column-partitioned; 2-buf rotation, so
            # chunk q+2 reuses chunk q's bank after its finalize)
            n_chunks = len(CHUNK_BOUNDS) - 1
            zt_ps = pssc.tile([128, 512], F32, tag="sc")  # z transpose target
            sc_ps = []
            for _q in range(n_chunks):
                sc_chunk = pssc.tile([128, 512], F32, tag="sc")
                sc_ps.append(sc_chunk)

            # PE warm-up burst while waiting for group 0's x: the HAM clock
            # gate needs ~3.4us of sustained matmul activity to lift the PE
            # from 1.2 to 2.4 GHz. A memset tile (no DMA dependency) lets
            # the burst start right after the preamble.
            wu_w = const.tile([128, 128], F16)
            nc.vector.memset(wu_w, 0.25)
            wu_ps = psgt.tile([D, 2, PAIRS_PER_GROUP, D], F32, tag="g2")
            for k in range(WARMUP_MM):
                nc.tensor.matmul(
                    out=wu_ps[:, 0, k % PAIRS_PER_GROUP, :],
                    lhsT=wu_w[:, 0:64],
                    rhs=wu_w[:, 64:128],
                    start=True,
                    stop=True,
                    tile_position=(0, 0),
                )

            # ---- x^2 pipeline, decoupled from the GEMM loop and paced by
            # x-group arrivals. DVE-only chain processing PAIRS of groups
            # per instruction (halves the per-op overhead; GPSIMD's folds
            # looked free but its SBUF traffic doubled DVE's square time).
            # ACT squares group 0 in its idle head window (its evictions
            # only start once the first GEMM finishes).
            PB = 2 * GB  # batches per pair
            def emit_sq_pair(k, defer_red=False):
                b0 = 2 * k * GB
                x2t = x2p.tile([128, PB, 2 * HW], F16, tag="x2", bufs=2)
                xflat = xall[:, b0 : b0 + PB, :, :].rearrange(
                    "p b par hw -> p b (par hw)"
                )
                if k == 0:
                    # group 0 on ACT (free until the first eviction),
                    # group 1 on DVE; both write halves of the pair tile
                    nc.scalar.activation(
                        out=x2t[:, 0:GB, :],
                        in_=xflat[:, 0:GB, :],
                        func=mybir.ActivationFunctionType.Square,
                    )
                    nc.vector.tensor_mul(
                        out=x2t[:, GB:PB, :],
                        in0=xflat[:, GB:PB, :],
                        in1=xflat[:, GB:PB, :],
                    )
                else:
                    nc.vector.tensor_mul(out=x2t[:, :, :], in0=xflat, in1=xflat)
                xh1 = x2p.tile([128, PB, HW], F16, tag="xh1", bufs=2)
                nc.vector.tensor_add(
                    out=xh1[:, :, :],
                    in0=x2t[:, :, 0:HW],
                    in1=x2t[:, :, HW : 2 * HW],
                )
                xh2 = x2p.tile([128, PB, HW // 2], F16, tag="xh2", bufs=2)
                nc.vector.tensor_add(
                    out=xh2[:, :, :],
                    in0=xh1[:, :, 0 : HW // 2],
                    in1=xh1[:, :, HW // 2 : HW],
                )
                if defer_red:
                    return xh2
                nc.vector.tensor_reduce(
                    out=z[:, b0 : b0 + PB],
                    in_=xh2[:, :, :],
                    axis=mybir.AxisListType.X,
                    op=mybir.AluOpType.add,
                )

            # the last pair's reduce is deferred until after the GEMM loop:
            # it then runs after DVE's share of the last evictions, so the
            # score phase is not gated on the x^2 chain's tail
            last_xh2 = None
            for k in range(NGROUPS // 2):
                last_xh2 = emit_sq_pair(k, defer_red=(k == NGROUPS // 2 - 1))

            # ---- main GEMM loop over batch groups (PE + ACT only).
            # GEMM is emitted one group ahead of Gt so the in-order PE queue
            # never stalls on ACT's ft eviction during pipeline fill.
            def emit_gemm(g, p0=0, npr=PAIRS_PER_GROUP):
                xv = xall[:, g * GB : (g + 1) * GB, :, :].rearrange(
                    "p (pr two) par hw -> p pr two par hw", two=2
                )[:, p0 : p0 + npr, :, :, :]
                ps2 = psmm.tile([128, 2, npr, HW], F32, tag="ps2")
                for fi, w0 in ((0, 0), (1, 2)):
                    for mi in range(4):
                        half, par0 = mi // 2, mi % 2
                        nc.tensor.matmul(
                            out=ps2[:, fi, :, :],
                            lhsT=wall[:, w0 + half, par0, :],
                            rhs=xv[:, :, half, par0, :],
                            start=(mi == 0),
                            stop=(mi == 3),
                        )
                ft_sb = ftp.tile([128, 2, npr, HW], F16, tag="ft")
                nc.scalar.copy(out=ft_sb[:, :, :, :], in_=ps2[:, :, :, :])
                return ft_sb

            def emit_gt(g, ft_sb, p0=0, npr=PAIRS_PER_GROUP, evict_dve=False):
                b0 = g * GB + 2 * p0
                # pair-packed Gt matmuls: Gt_b[j, i] = sum_d t[d,j] f[d,i]
                # The two row groups MUST write different psum banks:
                # concurrent row-tiled PE writes to one bank kill the HW run.
                # always allocate the full-size tile: a smaller one would
                # put the two quadrant row groups in the same psum bank,
                # and concurrent PE writes to one bank kill the HW run
                pgt2 = psgt.tile([D, 2, PAIRS_PER_GROUP, D], F32, tag="g2")
                for w in range(npr):
                    nc.tensor.matmul(
                        out=pgt2[:, 0, w, :],
                        lhsT=ft_sb[0:64, 1, w, :],
                        rhs=ft_sb[0:64, 0, w, :],
                        start=True,
                        stop=True,
                        tile_position=(0, 0),
                    )
                    nc.tensor.matmul(
                        out=pgt2[:, 1, w, :],
                        lhsT=ft_sb[64:128, 1, w, :],
                        rhs=ft_sb[64:128, 0, w, :],
                        start=True,
                        stop=True,
                        tile_position=(64, 0),
                    )
                # evict [j, (q, pair), i] -> gt[j, b, i], b = b0 + 2*pr + q
                # (contiguous 64-elem inner runs)
                ev = nc.vector.tensor_copy if evict_dve else nc.scalar.copy
                ev(
                    out=gt_sb[0:D, b0 : b0 + 2 * npr, :].rearrange(
                        "j (pr q) i -> j q pr i", q=2
                    ),
                    in_=pgt2[:, :, 0:npr, :],
                )
                # duplicate onto partitions 64:128 (idle sync queue) for
                # the double-pumped score phase
                nc.sync.dma_start(
                    out=gt_sb[D:128, b0 : b0 + 2 * npr, :],
                    in_=gt_sb[0:D, b0 : b0 + 2 * npr, :],
                )

            def keepalive(n, phase):
                # Dependency-free dummies that hold the HAM clock gate at
                # full speed while the PE waits on ACT's eviction chain.
                # They write unused columns of the zt tile (the only psum
                # slot with no pending rotation dependency).
                for k in range(n):
                    nc.tensor.matmul(
                        out=zt_ps[0:64, 128 + 64 * (k % 2) : 192 + 64 * (k % 2)],
                        lhsT=wu_w[:, 0:64],
                        rhs=wu_w[:, 64:128],
                        start=True,
                        stop=True,
                        tile_position=(0, 0),
                    )

            # group 7 is processed as two half-groups: the end-of-loop
            # serial chain (GEMM -> ft evict -> Gt -> gt evict -> score)
            # then only carries half a group's latency
            HP = PAIRS_PER_GROUP // 2
            fts = {}
            for g in range(NGROUPS - 1):
                fts[g] = emit_gemm(g)
                if g >= 1:
                    emit_gt(g - 1, fts[g - 1])
            ft7a = emit_gemm(NGROUPS - 1, 0, HP)
            emit_gt(NGROUPS - 2, fts[NGROUPS - 2])
            ft7b = emit_gemm(NGROUPS - 1, HP, HP)
            keepalive(KEEPALIVE_MM, 0)
            emit_gt(NGROUPS - 1, ft7a, 0, HP)
            emit_gt(NGROUPS - 1, ft7b, HP, HP)
            keepalive(KEEPALIVE_MM, 1)

            # deferred reduce of the last pair (after DVE's evict halves)
            nc.vector.tensor_reduce(
                out=z[:, B - PB : B],
                in_=last_xh2[:, :, :],
                axis=mybir.AxisListType.X,
                op=mybir.AluOpType.add,
            )

            # ---- one-hot score matmuls (columns in from_idx-sorted order).
            # Interleaved keepalive dummies hold the PE duty cycle high so
            # the HAM clock gate doesn't halve the clock mid-score (the
            # segments alone are LDWEIGHTS-paced at ~50% duty). They write
            # a fresh psgt-rotation tile, free once the last Gt eviction
            # has read its buffer.
            # segments grouped by chunk; hi=True runs a segment on PE
            # array rows 64:128 against the duplicated gt/onehot copies
            segs_by_chunk = [[] for _ in range(n_chunks)]
            for i, col0, ncols in seg_plan:
                q = next(
                    k for k in range(n_chunks) if CHUNK_BOUNDS[k + 1] > col0
                )
                segs_by_chunk[q].append((i, col0, ncols))

            def emit_seg(seg, hi):
                i, col0, ncols = seg
                q = next(
                    k for k in range(n_chunks) if CHUNK_BOUNDS[k + 1] > col0
                )
                c0 = col0 - CHUNK_BOUNDS[q]
                p0 = D if hi else 0
                nc.tensor.matmul(
                    out=sc_ps[q][:, c0 : c0 + ncols],
                    lhsT=gt_sb[p0 : p0 + D, :, i],
                    rhs=oh_sb[p0 : p0 + D, col0 : col0 + ncols],
                    start=True,
                    stop=True,
                    tile_position=(p0, 0),
                )

            def emit_fin(q):
                # fused finalize: out = score * inv[b] + promo_sorted
                q0 = CHUNK_BOUNDS[q]
                n = CHUNK_BOUNDS[q + 1] - q0
                nc.vector.scalar_tensor_tensor(
                    out=final_sb[:, q0 : q0 + n],
                    in0=sc_ps[q][:, 0:n],
                    scalar=inv_sb[:, 0:1],
                    in1=promo_bc[:, q0 : q0 + n],
                    op0=mybir.AluOpType.mult,
                    op1=mybir.AluOpType.add,
                )
                # per-chunk store so the DMA overlaps later chunks' finalize
                dma_eng = nc.sync if q % 2 == 0 else nc.scalar
                dma_eng.dma_start(
                    out=out[:, q0 : q0 + n],
                    in_=final_sb[:, q0 : q0 + n],
                )

            # chunk-0 segments first (single-pumped): they need only
            # gt + onehot, so they start right after the last Gt eviction
            # while the x^2 chain's tail (red-3) is still draining on DVE
            for seg in segs_by_chunk[0]:
                emit_seg(seg, False)

            # ---- 1/rms2 per batch (overlaps chunk 1+ score matmuls) ----
            # (transpose outputs must land at psum partition 0, so this
            # cannot be done incrementally per batch-slice)
            nc.tensor.transpose(out=zt_ps[:, 0:128], in_=z[:, :], identity=ident_sb[:, :])
            nc.vector.tensor_reduce(
                out=inv_sb[:, :],
                in_=zt_ps[:, 0:128],
                axis=mybir.AxisListType.X,
                op=mybir.AluOpType.add,
            )
            nc.vector.tensor_scalar(
                out=inv_sb[:, :],
                in0=inv_sb[:, :],
                scalar1=1.0 / (C * HW),
                scalar2=EPS,
                op0=mybir.AluOpType.mult,
                op1=mybir.AluOpType.add,
            )
            nc.vector.reciprocal(out=inv_sb[:, :], in_=inv_sb[:, :])
            emit_fin(0)

            # remaining chunks double-pumped in bank pairs: (1,2), (3,4).
            # The lo/hi streams write different psum banks, so the two PE
            # array halves run concurrently (same trick as the Gt matmuls).
            for qa, qb in ((1, 2), (3, 4)):
                la, lb = segs_by_chunk[qa], segs_by_chunk[qb]
                for k in range(max(len(la), len(lb))):
                    if k < len(la):
                        emit_seg(la[k], False)
                    if k < len(lb):
                        emit_seg(lb[k], True)
                emit_fin(qa)
                emit_fin(qb)

    nc.compile()
    return nc


_NC_CACHE = {}


def _plan_from_indices(from_idx, to_idx):
    from_idx = np.asarray(from_idx, np.int64)
    to_idx = np.asarray(to_idx, np.int64)
    order = np.argsort(from_idx, kind="stable")
    fi_sorted = from_idx[order]
    seg_plan = []
    col = 0
    for i in range(HW):
        n = int(np.count_nonzero(fi_sorted == i))
        while n > 0:
            # segments must not cross a psum chunk boundary
            bound = next(b for b in CHUNK_BOUNDS[1:] if b > col)
            m = min(n, bound - col)
            seg_plan.append((i, col, m))
            col += m
            n -= m
    assert col == V
    onehot = np.zeros((D, V), np.float16)
    onehot[to_idx[order], np.arange(V)] = 1.0
    return tuple(seg_plan), onehot, order


def _host_inputs(from_w, to_w):
    def stack_w(wmat):
        wt = np.ascontiguousarray(wmat.T).reshape(CP, 2, D)  # [cp, par, d]
        lo = np.zeros((2, CP, 128), np.float16)
        hi = np.zeros((2, CP, 128), np.float16)
        lo[:, :, 0:D] = wt.transpose(1, 0, 2)
        hi[:, :, D:128] = wt.transpose(1, 0, 2)
        return lo, hi

    wf_lo, wf_hi = stack_w(np.asarray(from_w, np.float32))
    wt_lo, wt_hi = stack_w(np.asarray(to_w, np.float32))
    return wf_lo, wf_hi, wt_lo, wt_hi


def _device_inputs(x, from_w, to_w, promo_bias, from_idx, to_idx, promo_idx):
    """Build (seg_plan, shared input map, per-core xs list, unsort order)."""
    seg_plan, onehot, order = _plan_from_indices(from_idx, to_idx)
    wf_lo, wf_hi, wt_lo, wt_hi = _host_inputs(from_w, to_w)
    # [4, 2, CP, 128] -> [CP, 4, 2, 128] contiguous so the upload DMA is
    # one 2KB descriptor per partition
    wpack = np.ascontiguousarray(
        np.stack([wf_lo, wf_hi, wt_lo, wt_hi], axis=0).transpose(2, 0, 1, 3)
    )
    promo = np.asarray(promo_bias, np.float32)[np.asarray(promo_idx, np.int64)]
    shared = {
        "wpack": wpack,
        "ident": np.eye(128, dtype=np.float32),
        "onehot": np.ascontiguousarray(onehot),
        "promo": promo[order].astype(np.float16)[None, :],
    }
    # x [B_TOT, C, HW] -> per-core [cp, b, par, hw] fp16 (4KB contiguous
    # per partition per group DMA)
    xr = np.asarray(x, np.float32).reshape(B_TOT, C, HW)
    xs_list = []
    for c in range(N_CORES):
        xc = xr[c * B : (c + 1) * B].reshape(B, CP, 2, HW)
        xs_list.append(np.ascontiguousarray(xc.transpose(1, 0, 2, 3)).astype(np.float16))
    return seg_plan, shared, xs_list, order


def kernel(
    x,
    norm_weight,
    from_w,
    from_b,
    to_w,
    to_b,
    promo_bias,
    from_idx,
    to_idx,
    promo_idx,
):
    x = np.asarray(x, np.float32)
    norm_weight = np.asarray(norm_weight, np.float32)
    from_b = np.asarray(from_b, np.float32)
    to_b = np.asarray(to_b, np.float32)

    if (
        np.any(from_b != 0.0)
        or np.any(to_b != 0.0)
        or not np.allclose(norm_weight, 1.0)
    ):
        # General-correctness fallback; never hit for this problem's input
        # distribution (norm_weight is ones, conv biases are zeros).
        return _host_reference(
            x, norm_weight, from_w, from_b, to_w, to_b, promo_bias,
            from_idx, to_idx, promo_idx,
        )

    seg_plan, shared, xs_list, order = _device_inputs(
        x, from_w, to_w, promo_bias, from_idx, to_idx, promo_idx
    )
    if seg_plan not in _NC_CACHE:
        _NC_CACHE[seg_plan] = build_kernel(seg_plan)
    nc = _NC_CACHE[seg_plan]

    in_maps = [dict(shared, xs=xs_list[c]) for c in range(N_CORES)]
    res = run_bass_kernel_spmd(nc, in_maps, core_ids=list(range(N_CORES)))
    full = np.empty((B_TOT, V), np.float32)
    for c in range(N_CORES):
        dev = np.asarray(res.results[c]["out"], np.float32)  # sorted columns
        full[c * B : (c + 1) * B, order] = dev
    return full


def _host_reference(
    x, norm_weight, from_w, from_b, to_w, to_b, promo_bias, from_idx, to_idx, promo_idx
):
    b, c, w, h = x.shape
    rms = np.sqrt(np.mean(x * x, axis=(1, 2, 3), keepdims=True) + EPS)
    xn = (x / rms) * norm_weight[None]
    f = (
        np.einsum("bchw,dc->bdhw", xn, from_w) + from_b[None, :, None, None]
    ).reshape(b, -1, w * h)
    t = (
        np.einsum("bchw,dc->bdhw", xn, to_w) + to_b[None, :, None, None]
    ).reshape(b, -1, w * h)
    score = np.einsum("bdv,bdv->bv", f[:, :, from_idx], t[:, :, to_idx])
    return (score + promo_bias[promo_idx][None, :]).astype(np.float32)
